# revision 4
# baseline (speedup 1.0000x reference)
"""LocalitySelfAttention TRN2 kernel (v1: warm-PE pipeline + row-tiled scores).

B=4, N=2048, C=768, H=12, D=64.  8 cores: core c -> batch c//2, heads
6*(c%2) .. 6*(c%2)+6 (6 contiguous heads = 3 pairs).  Each core computes its
heads' qkv projection, attention, and a partial output projection restricted
to its heads' 384 rows of w_proj.  Host sums the two partials per batch and
adds b_proj.

Phase-2 dataflow per head-pair (heads 2p at partitions 0-63, 2p+1 at 64-127
of the qkT tile):
  - scores: two K=64 matmuls issued back-to-back at tile_position (0,0) and
    (64,0) so the PE runs them concurrently on disjoint row groups.
  - exp on ScalarE per head per [128 keys, 1024 q] PSUM tile (double
    buffered so ACT streams continuously; ACT is the steady-state bound).
  - AV: lhsT = v_aug [128 keys, 64+1(ones)], rhs = exp tile -> [65, q]
    accumulated over the 16 key blocks; row 64 = softmax denominators.
  - normalize on DVE with a DRAM-bounce partition broadcast of 1/denom.
Keeping the PE fed continuously avoids the HAM half-clock throttle that
dominated the previous version.
"""

import sys
import numpy as np

if "/opt/trn_rl_repo" not in sys.path:
    sys.path.insert(0, "/opt/trn_rl_repo")

B, N, C, H = 4, 2048, 768, 12
D = C // H          # 64
NH = 6              # heads per core
NP = NH // 2        # head pairs per core = 3
P = 128
CT = C // P         # 6 contraction tiles
KB = N // P         # 16 key blocks
QC = N // 512       # 4 free-dim chunks of 512
HF = 1024           # q processed in halves
SCALE = float(D) ** -0.5  # 0.125

_CACHE = {}


def _build_program():
    import concourse.bass as bass
    import concourse.mybir as mybir
    import concourse.tile as tile
    from concourse import bacc
    from concourse.masks import make_identity

    f32 = mybir.dt.float32
    f32r = mybir.dt.float32r
    bf16 = mybir.dt.bfloat16
    Exp = mybir.ActivationFunctionType.Exp
    mult = mybir.AluOpType.mult
    add = mybir.AluOpType.add

    nc = bacc.Bacc()
    xT = nc.dram_tensor("xT", [C, N], f32r, kind="ExternalInput")
    wqkv = nc.dram_tensor("wqkv", [C, 3 * NH * D], f32r, kind="ExternalInput")
    wproj = nc.dram_tensor("wproj", [NH * D, C], f32r, kind="ExternalInput")
    temp = nc.dram_tensor("temp", [P, NH], f32, kind="ExternalInput")
    outT = nc.dram_tensor("outT", [C, N], f32, kind="ExternalOutput")
    rdram = nc.dram_tensor("rscratch", [2 * NP * 2, HF], f32)  # recip rows

    def mm(out, lhsT, rhs, **kw):
        if lhsT.dtype == f32:
            lhsT = lhsT.bitcast(f32r)
        if rhs.dtype == f32:
            rhs = rhs.bitcast(f32r)
        nc.tensor.matmul(out, lhsT, rhs, **kw)

    with tile.TileContext(nc) as tc:
        with (
            tc.tile_pool(name="const", bufs=1) as constp,
            tc.tile_pool(name="persist", bufs=1) as persist,
        ):
            # ---- setup: temperature diag masks (1 - t_h * I) ----------
            ident = constp.tile([P, P], f32, tag="ident")
            make_identity(nc, ident[:])
            tbc = constp.tile([P, NH], f32, tag="tbc")
            nc.sync.dma_start(tbc[:, :], temp[:, :])
            ntb = constp.tile([P, NH], f32, tag="ntb")
            nc.vector.tensor_scalar_mul(ntb[:, :], tbc[:, :], -1.0)
            masks = constp.tile([P, NH, P], f32, tag="masks")
            for h in range(NH):
                nc.vector.tensor_scalar(
                    masks[:, h, :], ident[:], ntb[:, h : h + 1], 1.0, mult, add
                )

            # persistent: qT/kT (head pairs stacked on partitions), v_aug
            qkT = persist.tile([P, 2 * NP, N], bf16, tag="qkT")  # 0-2 q, 3-5 k
            vaug = persist.tile([P, KB, NH, D + 1], bf16, tag="vaug")
            onesrc = constp.tile([P, KB * NH], f32, tag="onesrc")
            nc.vector.memset(onesrc[:], 1.0)
            nc.vector.tensor_copy(
                vaug[:, :, :, D : D + 1],
                onesrc[:].rearrange("p (a b c) -> p a b c", a=KB, b=NH),
            )
            wp = persist.tile([P, NH * D // P, C], bf16, tag="wp")  # [128,3,768]
            attnT = persist.tile([P, NP, N], bf16, tag="attnT")

            # ---- phase 1: qkv projection ------------------------------
            with (
                tc.tile_pool(name="qin", bufs=1) as qin,
                tc.tile_pool(name="psum1", bufs=2, space=bass.MemorySpace.PSUM) as psum1,
            ):
                xts, wqs = [], []
                for t in range(CT):
                    xti = qin.tile([P, N], f32r, tag=f"xt{t}")
                    nc.sync.dma_start(xti[:], xT[t * P : (t + 1) * P, :])
                    xts.append(xti)
                    wqi = qin.tile([P, 3 * NH * D], f32r, tag=f"wq{t}")
                    nc.sync.dma_start(wqi[:], wqkv[t * P : (t + 1) * P, :])
                    wqs.append(wqi)
                wps = qin.tile([P, NH * D // P, C], f32r, tag="wps")
                for g3 in range(NH * D // P):
                    nc.sync.dma_start(wps[:, g3, :], wproj[g3 * P : (g3 + 1) * P, :])
                nc.vector.tensor_copy(wp[:], wps[:])

                # q,k transposed: 6 groups of 128 cols (3 q head-pairs, 3 k)
                for g in range(6):
                    ps = psum1.tile([P, N], f32, tag="ps")
                    for t in range(CT):
                        for qc in range(QC):
                            mm(
                                ps[:, qc * 512 : (qc + 1) * 512],
                                wqs[t][:, g * P : (g + 1) * P],
                                xts[t][:, qc * 512 : (qc + 1) * 512],
                                start=(t == 0),
                                stop=(t == CT - 1),
                            )
                    nc.vector.tensor_copy(qkT[:, g, :], ps[:])

                # v natural, interleaved with ones column
                for rb_i in range(KB):
                    psv = psum1.tile([P, NH * D], f32, tag="ps")
                    for t in range(CT):
                        mm(
                            psv[:],
                            xts[t][:, rb_i * P : (rb_i + 1) * P],
                            wqs[t][:, 2 * NH * D : 3 * NH * D],
                            start=(t == 0),
                            stop=(t == CT - 1),
                        )
                    nc.vector.tensor_copy(
                        vaug[:, rb_i, :, 0:D],
                        psv[:].rearrange("p (h d) -> p h d", h=NH),
                    )

            # ---- phase 2: attention, pair-pipelined -------------------
            with (
                tc.tile_pool(name="pt", bufs=4) as ptp,
                tc.tile_pool(name="un", bufs=4) as unp,
                tc.tile_pool(name="rb", bufs=4) as rbp,
                tc.tile_pool(name="pst", bufs=2, space=bass.MemorySpace.PSUM) as pst,
                tc.tile_pool(name="pav", bufs=2, space=bass.MemorySpace.PSUM) as pav,
            ):
                for p in range(NP):
                    hA, hB = 2 * p, 2 * p + 1
                    for hf in range(2):
                        q0 = hf * HF
                        avA = pav.tile([D + 1, HF], f32, tag="av", name=f"avA{p}_{hf}")
                        avB = pav.tile([D + 1, HF], f32, tag="av", name=f"avB{p}_{hf}")
                        for kb in range(KB):
                            stA = pst.tile([P, HF], f32, tag="st")
                            stB = pst.tile([P, HF], f32, tag="st")
                            for qc in range(2):
                                cs = slice(qc * 512, (qc + 1) * 512)
                                qs = slice(q0 + qc * 512, q0 + (qc + 1) * 512)
                                ks = slice(kb * P, (kb + 1) * P)
                                mm(stA[:, cs], qkT[0:D, NP + p, ks],
                                   qkT[0:D, p, qs], start=True, stop=True)
                                mm(stB[:, cs], qkT[D:P, NP + p, ks],
                                   qkT[D:P, p, qs], start=True, stop=True)
                            if kb * P // HF == hf:
                                dcol = kb * P - q0
                                dsl = slice(dcol, dcol + P)
                                nc.vector.tensor_mul(
                                    stA[:, dsl], stA[:, dsl], masks[:, hA, :]
                                )
                                nc.vector.tensor_mul(
                                    stB[:, dsl], stB[:, dsl], masks[:, hB, :]
                                )
                            ptA = ptp.tile([P, HF], bf16, tag="pt")
                            nc.scalar.activation(ptA[:], stA[:], Exp, scale=SCALE)
                            ptB = ptp.tile([P, HF], bf16, tag="pt")
                            nc.scalar.activation(ptB[:], stB[:], Exp, scale=SCALE)
                            for qc in range(2):
                                cs = slice(qc * 512, (qc + 1) * 512)
                                mm(avA[:, cs], vaug[:, kb, hA, :], ptA[:, cs],
                                   start=(kb == 0), stop=(kb == KB - 1))
                                mm(avB[:, cs], vaug[:, kb, hB, :], ptB[:, cs],
                                   start=(kb == 0), stop=(kb == KB - 1))
                        # normalize: attnT rows = av[0:64] * recip(av[64])
                        for off, avX, h in ((0, avA, hA), (D, avB, hB)):
                            un = unp.tile([D + 1, HF], f32, tag="un")
                            nc.vector.tensor_copy(un[:], avX[:])
                            nc.vector.reciprocal(
                                un[D : D + 1, :], un[D : D + 1, :]
                            )
                            ri = 2 * h + hf
                            nc.sync.dma_start(rdram[ri, :], un[D : D + 1, :])
                            rb = rbp.tile([D, HF], f32, tag="rb")
                            nc.sync.dma_start(
                                rb[:], rdram[ri : ri + 1, :].broadcast_to([D, HF])
                            )
                            nc.vector.tensor_mul(
                                attnT[off : off + D, p, q0 : q0 + HF],
                                un[0:D, :],
                                rb[:],
                            )

            # ---- phase 3: output projection (transposed) --------------
            with (
                tc.tile_pool(name="psum3", bufs=2, space=bass.MemorySpace.PSUM) as psum3,
                tc.tile_pool(name="ot", bufs=2) as otp,
            ):
                for m in range(CT):
                    po = psum3.tile([P, N], f32, tag="ps")
                    for g3 in range(NH * D // P):
                        for qc in range(QC):
                            cs = slice(qc * 512, (qc + 1) * 512)
                            mm(
                                po[:, cs],
                                wp[:, g3, m * P : (m + 1) * P],
                                attnT[:, g3, cs],
                                start=(g3 == 0),
                                stop=(g3 == NH * D // P - 1),
                            )
                    ot = otp.tile([P, N], f32, tag="ot")
                    nc.vector.tensor_copy(ot[:], po[:])
                    nc.sync.dma_start(outT[m * P : (m + 1) * P, :], ot[:])

    if not nc.is_finalized():
        nc.finalize()
    return nc


def _get_program():
    if "nc" not in _CACHE:
        _CACHE["nc"] = _build_program()
    return _CACHE["nc"]


def _in_maps(x, w_qkv, w_proj, temperature):
    t = np.asarray(temperature, dtype=np.float32).reshape(H)
    maps = []
    xTs = {}
    for c in range(8):
        b, h0 = c // 2, NH * (c % 2)
        if b not in xTs:
            xTs[b] = np.ascontiguousarray(np.asarray(x[b], dtype=np.float32).T)
        cols = slice(D * h0, D * h0 + NH * D)
        wq = np.concatenate(
            [w_qkv[:, cols], w_qkv[:, C:][:, cols], w_qkv[:, 2 * C :][:, cols]],
            axis=1,
        )
        maps.append(
            {
                "xT": xTs[b],
                "wqkv": np.ascontiguousarray(wq, dtype=np.float32),
                "wproj": np.ascontiguousarray(
                    w_proj[D * h0 : D * h0 + NH * D, :], dtype=np.float32
                ),
                "temp": np.ascontiguousarray(
                    np.broadcast_to(t[h0 : h0 + NH].reshape(1, NH), (P, NH))
                ),
            }
        )
    return maps


def _install_profile_hook():
    """The agent image's antenv lacks axon_hooks; synthesize it and register
    the ctypes NTFF hook so run_bass_kernel_spmd(trace=True) can profile."""
    import types, importlib

    if "antenv.axon_hooks" not in sys.modules:
        import antenv

        mod = types.ModuleType("antenv.axon_hooks")
        _state = {"hook": None}
        mod.set_axon_ntff_profile_hook = lambda h: _state.__setitem__("hook", h)
        mod.get_axon_ntff_profile_hook = lambda: _state["hook"]
        sys.modules["antenv.axon_hooks"] = mod
        antenv.axon_hooks = mod
    from antenv.axon_hooks import (
        get_axon_ntff_profile_hook,
        set_axon_ntff_profile_hook,
    )

    if get_axon_ntff_profile_hook() is None:
        tb = importlib.import_module("trn_agent_boot.trn_boot")
        hook = tb._ntff_profile_via_ctypes("/opt/axon/libaxon_pjrt.so")
        set_axon_ntff_profile_hook(hook)


def kernel(x, w_qkv, w_proj, b_proj, temperature, _trace=False):
    from concourse.bass_utils import run_bass_kernel_spmd

    if _trace:
        try:
            _install_profile_hook()
        except Exception as e:  # profiling is best-effort
            print(f"profile hook install failed: {e}")

    nc = _get_program()
    maps = _in_maps(
        np.asarray(x, np.float32),
        np.asarray(w_qkv, np.float32),
        np.asarray(w_proj, np.float32),
        np.asarray(temperature, np.float32),
    )
    res = run_bass_kernel_spmd(nc, maps, list(range(8)), trace=_trace)
    parts = [r["outT"] for r in res.results]
    bp = np.asarray(b_proj, np.float32)
    out = np.stack(
        [(parts[2 * b] + parts[2 * b + 1]).T + bp for b in range(B)]
    ).astype(np.float32)
    if _trace:
        _CACHE["last_result"] = res
    return out
